# revision 27
# baseline (speedup 1.0000x reference)
"""AdaProj kernel for 8 TRN2 NeuronCores (baseline schedule + SWDGE-
prepared output writeback).

Math: per class c, sample b:
  L_s[c,b] = W[c,s,:] . x[b,:]   (raw matmul)
  rnw[c,s] = 1/||W[c,s,:]||, rnx[b] = 1/||x[b]||
  m_s = rnw_s * L_s
  num = sum_s m_s^2
  den = num + sum_{s<s'} h_ss' * m_s * m_s',  h_ss' = 2*Graw_ss'*rnw_s*rnw_s'
  out[c,b] = rnx_b * num / sqrt(den)

Structure:
  - x and W packed into ONE dram tensor xw [D, 256+500] fp16, loaded in 4
    contiguous k-chunks so matmuls start on chunk 0 while later chunks
    stream.
  - PE warmed with dummy matmuls during the DMA wait (p-state ramp).
  - All sum-over-D reductions (W norms, Gram pairs) are PE matmuls with
    free-size-1 outputs: lhsT = product chunk [128, <=125], rhs = ones.
  - num and the cross term accumulate in PSUM via identity-matmuls
    (lhsT = I_125) over the fp16 product tiles, freeing DVE adds.
  - Per-k product work split: W-squares on Act, pair products on DVE,
    (0,3) pair on gpsimd; xsq on DVE.
  - Output via kv_writeback: SWDGE descriptors generated right after the
    final ot write (Pool), fired by trigger_dma — the transfer skips the
    HWDGE descriptor stage (625ns) and DGE ramp (650ns) on the tail.

Sharding: W split over classes C (125/core); x replicated; host
concatenates the per-core [125, 256] outputs.
"""

import numpy as np

import concourse.bacc as bacc
import concourse.bass as bass
import concourse.mybir as mybir
import concourse.tile as tile
from concourse.bass_utils import run_bass_kernel_spmd

B, C, S, D = 256, 1000, 4, 512
NCORES = 8
CS = C // NCORES          # 125 classes per core
R = CS * S                # 500 W rows per core
KP = D // 128             # 4 contraction chunks
XW = B + R                # 756 packed columns: [x | w]

F32 = mybir.dt.float32
FP16 = mybir.dt.float16
I32 = mybir.dt.int32
AF = mybir.ActivationFunctionType
OP = mybir.AluOpType

N_WARM = 20  # dummy PE matmuls holding the p-state ramp until data lands

_CACHED = {}


def _emit_body(nc, pool, psum):
    xw_d = nc.dram_tensor("xw", [D, XW], FP16, kind="ExternalInput")
    out_d = nc.dram_tensor("out", [1, 128, 1, B], FP16, kind="ExternalOutput")

    def st(shape, dtype, name, space_pool=None):
        sp = space_pool if space_pool is not None else pool
        return sp.tile(shape, dtype, tag=name, name=name)

    def mm(out, lhsT, rhs, start, stop):
        return nc.tensor.matmul(out, lhsT, rhs, start=start, stop=stop,
                                skip_group_check=True)

    def mm_chain(prev, out, lhsT, rhs, start, stop):
        i = mm(out, lhsT, rhs, start, stop)
        if prev is not None:
            bass._add_dep_helper(i.ins, prev.ins, sync=False,
                                 reason="psum accumulation group order")
        return i

    # ---------------- tiny init + act table warm ----------------
    warm = st([1, 1], F32, "warm")
    nc.vector.memset(warm[:], 1.0)
    warm3 = st([1, 1], F32, "warm3")
    nc.scalar.activation(warm3[:], warm[:], AF.Abs_reciprocal_sqrt)

    ones_w = st([128, 1], FP16, "ones_w")
    nc.vector.memset(ones_w[:], 1.0)
    dum = st([128, 128], FP16, "dum")
    nc.vector.memset(dum[:], 0.03)
    ones_row = st([1, 128], FP16, "ones_row")
    nc.vector.memset(ones_row[:], 1.0)
    # identity [125,125] fp16 for the PSUM-accumulating identity matmuls
    eye = st([CS, CS], FP16, "eye")
    nc.vector.memset(eye[:], 1.0)
    nc.gpsimd.affine_select(
        eye[:], eye[:], pattern=[[-1, CS]], compare_op=OP.is_equal,
        fill=0.0, base=0, channel_multiplier=1,
    )
    ctx0 = st([128, 1], I32, "ctx0")
    nc.vector.memset(ctx0[:], 0)
    ot = st([128, 1, 1, B], FP16, "ot")
    nc.vector.memset(ot[:], 0.0)

    # ---------------- PE warmup (writes the numb bank, reset later) -----
    numb = st([CS, B], F32, "numb", psum)
    nx = numb[0:1, :]
    for i in range(N_WARM):
        mm(nx[:, 0:128], ones_w[:], dum[:], start=True, stop=True)

    # ---------------- input DMAs (SP HWDGE, 4 k-chunks) ----------------
    xw = st([128, KP, XW], FP16, "xw")
    nc.gpsimd.dma_start(xw[:, 3, :], xw_d[3 * 128:4 * 128, :])
    for k in range(KP - 1):
        nc.sync.dma_start(xw[:, k, :], xw_d[k * 128:(k + 1) * 128, :])

    def wsl(k, lo, hi):
        return xw[:, k, B + lo:B + hi]

    # ---------------- PE: L matmuls, k-major (packed banks) -------------
    Lp = [st([CS, B], F32, f"L{s}", psum) for s in range(S)]
    Lsl = [Lp[s][:] for s in range(S)]
    KORD = [0, 3, 1, 2]
    Lmm = [None] * S
    for ki, k in enumerate(KORD):
        for s in range(S):
            Lmm[s] = mm_chain(Lmm[s], Lsl[s], wsl(k, s * CS, (s + 1) * CS),
                              xw[:, k, 0:B], start=(ki == 0), stop=(ki == KP - 1))

    # ---------------- per-k products: Act squares, DVE pairs, Pool (0,3)
    prodD = st([128, KP, R], FP16, "prodD")
    prodA = st([128, KP, 3 * CS], FP16, "prodA")   # (0,1),(1,2),(2,3)
    prodB = st([128, KP, 2 * CS], FP16, "prodB")   # (0,2),(1,3)
    prodC = st([128, KP, CS], FP16, "prodC")       # (0,3)
    xsq = st([128, KP, B], FP16, "xsq")
    KLAST = KORD[-1]   # last-arriving chunk (k2)
    for k in KORD:
        if k != KLAST:
            nc.scalar.activation(prodD[:, k, :], wsl(k, 0, R), AF.Square)
            nc.vector.tensor_tensor(prodA[:, k, :], wsl(k, 0, 3 * CS), wsl(k, CS, R), OP.mult)
            nc.vector.tensor_tensor(prodB[:, k, :], wsl(k, 0, 2 * CS), wsl(k, 2 * CS, R), OP.mult)
            nc.gpsimd.tensor_tensor(xsq[:, k, :], xw[:, k, 0:B], xw[:, k, 0:B], OP.mult)
        else:
            pd3b = nc.vector.tensor_tensor(prodD[:, k, :], wsl(k, 0, R), wsl(k, 0, R), OP.mult)
        nc.gpsimd.tensor_tensor(prodC[:, k, :], wsl(k, 0, CS), wsl(k, 3 * CS, R), OP.mult)

    # ---------------- PE: norm/gram reductions (free-size-1 matmuls) ----
    nsqx = st([CS, 8 + B], F32, "nsqx", psum)
    nsq = nsqx[:, 0:S]
    gqx = st([CS, 6], F32, "gqx", psum)
    gq = gqx[:, 0:6]
    Nmm = [None] * S
    for ki, k in enumerate(KORD):
        for s in range(S):
            Nmm[s] = mm_chain(Nmm[s], nsqx[:, s:s + 1],
                              prodD[:, k, s * CS:(s + 1) * CS], ones_w[:],
                              start=(ki == 0 and s == 0), stop=(ki == KP - 1))
            if ki == 0 and s > 0:
                bass._add_dep_helper(Nmm[s].ins, Nmm[0].ins, sync=False,
                                     reason="bank opener first")
    last_nsq = Nmm[S - 1]
    # ---------------- rnw + m copies ----------------
    rnw = st([CS, S], F32, "rnw")
    nc.scalar.activation(rnw[:], nsq, AF.Abs_reciprocal_sqrt)
    m = st([CS, S, B], FP16, "m")
    m0i = nc.scalar.mul(m[:, 0, :], Lsl[0], rnw[:, 0:1])
    nc.vector.tensor_scalar_mul(m[:, 1, :], Lsl[1], rnw[:, 1:2])
    m2i = nc.scalar.mul(m[:, 2, :], Lsl[2], rnw[:, 2:3])
    m3i = nc.vector.tensor_scalar_mul(m[:, 3, :], Lsl[3], rnw[:, 3:4])
    # deferred last-chunk products (gram + rnx inputs, non-critical; after prodD)
    pa3 = nc.vector.tensor_tensor(prodA[:, KLAST, :], wsl(KLAST, 0, 3 * CS), wsl(KLAST, CS, R), OP.mult)
    bass._add_dep_helper(pa3.ins, pd3b.ins, sync=False, reason="prodD first")
    nc.vector.tensor_tensor(prodB[:, KLAST, :], wsl(KLAST, 0, 2 * CS), wsl(KLAST, 2 * CS, R), OP.mult)
    nc.vector.tensor_tensor(xsq[:, KLAST, :], xw[:, KLAST, 0:B], xw[:, KLAST, 0:B], OP.mult)

    # gram + rnx reductions (need all-k products)
    Gmm = [None] * 6
    for ki, k in enumerate(KORD):
        for j in range(3):
            Gmm[j] = mm_chain(Gmm[j], gqx[:, j:j + 1],
                              prodA[:, k, j * CS:(j + 1) * CS], ones_w[:],
                              start=(ki == 0 and j == 0), stop=(ki == KP - 1))
            if ki == 0 and j > 0:
                bass._add_dep_helper(Gmm[j].ins, Gmm[0].ins, sync=False,
                                     reason="bank opener first")
        for j in range(2):
            Gmm[3 + j] = mm_chain(Gmm[3 + j], gqx[:, 3 + j:4 + j],
                                  prodB[:, k, j * CS:(j + 1) * CS], ones_w[:],
                                  start=False, stop=(ki == KP - 1))
            if ki == 0:
                bass._add_dep_helper(Gmm[3 + j].ins, Gmm[0].ins, sync=False,
                                     reason="bank opener first")
        Gmm[5] = mm_chain(Gmm[5], gqx[:, 5:6], prodC[:, k, :], ones_w[:],
                          start=False, stop=(ki == KP - 1))
        if ki == 0:
            bass._add_dep_helper(Gmm[5].ins, Gmm[0].ins, sync=False,
                                 reason="bank opener first")

    # ---------------- rnx reduction on PE (reuses warm bank) ------------
    nxi = None
    for k in range(KP):
        nxi = mm_chain(nxi, nx, ones_w[:], xsq[:, k, :],
                       start=(k == 0), stop=(k == KP - 1))
        if k == 0:
            bass._add_dep_helper(nxi.ins, last_nsq.ins, sync=False,
                                 reason="norm reductions first on PE")

    # rnx row + broadcast (ordering hint keeps it behind m0 on Act)
    rnx_row = st([1, B], FP16, "rnx_row")
    rri = nc.scalar.activation(rnx_row[:], nx, AF.Abs_reciprocal_sqrt)
    bass._add_dep_helper(rri.ins, m2i.ins, sync=False,
                         reason="m copies first on Act")
    rnx_bc = nsqx[:, 8:8 + B]
    bci = mm(rnx_bc, ones_row[:, 0:CS], rnx_row[:], start=False, stop=True)
    bass._add_dep_helper(bci.ins, Nmm[0].ins, sync=False,
                         reason="bank opener first")

    # ---------------- gram coefficients ----------------
    t6 = st([CS, 6], F32, "t6")
    nc.vector.tensor_tensor(t6[:, 0:3], rnw[:, 0:3], rnw[:, 1:4], OP.mult)
    nc.vector.tensor_tensor(t6[:, 3:5], rnw[:, 0:2], rnw[:, 2:4], OP.mult)
    nc.vector.tensor_tensor(t6[:, 5:6], rnw[:, 0:1], rnw[:, 3:4], OP.mult)
    h = st([CS, 6], F32, "h")
    nc.vector.scalar_tensor_tensor(
        out=h[:], in0=gq, scalar=2.0, in1=t6[:], op0=OP.mult, op1=OP.mult,
    )

    # ---------------- epilogue products + scaled cross terms ------------
    Q01 = st([CS, 2, B], FP16, "Q01")
    nc.vector.tensor_tensor(Q01[:], m[:, 0:2, :], m[:, 0:2, :], OP.mult)
    Q23 = st([CS, 2, B], FP16, "Q23")
    nc.vector.tensor_tensor(Q23[:], m[:, 2:4, :], m[:, 2:4, :], OP.mult)
    psA = st([CS, 3, B], FP16, "psA")
    nc.vector.tensor_tensor(psA[:], m[:, 0:3, :], m[:, 1:4, :], OP.mult)
    cpA = st([CS, 3, B], FP16, "cpA")
    nc.vector.tensor_scalar_mul(cpA[:, 0, :], psA[:, 0, :], h[:, 0:1])
    nc.vector.tensor_scalar_mul(cpA[:, 1, :], psA[:, 1, :], h[:, 1:2])
    nc.scalar.mul(cpA[:, 2, :], psA[:, 2, :], h[:, 2:3])
    psB = st([CS, 2, B], FP16, "psB")
    nc.vector.tensor_tensor(psB[:], m[:, 0:2, :], m[:, 2:4, :], OP.mult)
    cpB = st([CS, 2, B], FP16, "cpB")
    nc.vector.tensor_scalar_mul(cpB[:, 0, :], psB[:, 0, :], h[:, 3:4])
    nc.vector.tensor_scalar_mul(cpB[:, 1, :], psB[:, 1, :], h[:, 4:5])
    # pair (0,3) on gpsimd
    psC = st([CS, B], FP16, "psC")
    nc.gpsimd.tensor_tensor(psC[:], m[:, 0, :], m[:, 3, :], OP.mult)
    cpC = st([CS, B], FP16, "cpC")
    nc.gpsimd.tensor_scalar_mul(cpC[:], psC[:], h[:, 5:6])

    # ---------------- num & den accumulation on PE (identity matmuls) ---
    Qsl = [Q01[:, 0, :], Q01[:, 1, :], Q23[:, 0, :], Q23[:, 1, :]]
    nmm = None
    for s in range(S):
        nmm = mm_chain(nmm, numb[:], eye[:], Qsl[s],
                       start=(s == 0), stop=(s == S - 1))
    denb = st([CS, B], F32, "denb", psum)
    dmm = None
    for s in range(S):
        dmm = mm_chain(dmm, denb[:], eye[:], Qsl[s],
                       start=(s == 0), stop=False)
    dmm = mm_chain(dmm, denb[:], eye[:], cpC[:], start=False, stop=False)
    for j in range(3):
        dmm = mm_chain(dmm, denb[:], eye[:], cpA[:, j, :], start=False, stop=False)
    for j in range(2):
        dmm = mm_chain(dmm, denb[:], eye[:], cpB[:, j, :], start=False, stop=(j == 1))

    # u = num * rnx (off critical path)
    rnx_sb = st([CS, B], FP16, "rnx_sb")
    nc.scalar.copy(rnx_sb[:], rnx_bc)
    u = st([CS, B], FP16, "u")
    nc.vector.tensor_tensor(u[:], numb[:], rnx_sb[:], OP.mult)

    srd = st([CS, B], FP16, "srd")
    nc.scalar.activation(srd[:], denb[:], AF.Abs_reciprocal_sqrt)
    nc.vector.tensor_tensor(ot[0:CS, 0, 0, :], u[:], srd[:], OP.mult)

    # output: SWDGE descriptors prepared after the ot write (the simulator
    # reads the source at prep position), fired by trigger_dma — skips the
    # HWDGE descriptor stage + DGE ramp on the critical tail.
    sem_ot = nc.alloc_semaphore("dma_ot")
    nc.gpsimd.kv_writeback(
        out_d[:, :, :, :], ot[:, :, :, :], ctx0[:],
        prepare_only=True, sem=sem_ot,
    )
    nc.gpsimd.trigger_dma(count=None)


def _build_nc():
    nc = bacc.Bacc(
        "TRN2",
        target_bir_lowering=False,
        debug=False,
        enable_asserts=False,
        num_devices=NCORES,
    )
    with tile.TileContext(nc) as tc:
        with (
            tc.tile_pool(name="main", bufs=1) as pool,
            tc.tile_pool(name="psum", bufs=1, space="PSUM") as psum,
        ):
            _emit_body(nc, pool, psum)
    nc.compile()
    _fix_swdge_waits(nc)
    return nc


def _fix_swdge_waits(nc):
    """Point consumer waits at the sems the SWDGE descriptors actually fire.

    Tile routes data deps on prepared-SWDGE outputs through per-lane DMASW<i>
    semaphores, but the hardware descriptor encodes exactly one sem — the
    user sem passed via ``sem=`` (on_update[0], +16 at transfer end). Tile
    never attaches the DMASW increment for gen_mode==1 preps, leaving those
    DMASW waits unsatisfiable. Rewrite each unsatisfied ``DMASW<i> >= 16``
    wait to the user sem of the prep on that lane (lanes assigned
    round-robin over Pool DMA instructions in program order).
    """
    import re

    fn = nc.m.functions[0]
    lane_sem = {}
    updated = set()
    n_dma = 0
    for blk in fn.blocks:
        for ins in blk.instructions:
            if ins.sync_info is None:
                continue
            for upd in ins.sync_info.on_update:
                if upd.ant_name:
                    updated.add(upd.ant_name)
            if ins.engine == mybir.EngineType.Pool and (
                    type(ins).__name__ in ("InstDMACopy", "InstDMAGatherAnt",
                                           "InstKVWritebackAnt",
                                           "InstDMAScatterAddAnt",
                                           "InstPagedWritebackAnt")):
                if getattr(ins, "gen_mode", 0) == 1:
                    lane_sem[n_dma] = ins.sync_info.on_update[0]
                n_dma += 1
    for blk in fn.blocks:
        for ins in blk.instructions:
            if ins.sync_info is None:
                continue
            for w in ins.sync_info.on_wait:
                m = re.match(r"DMASW(\d+)_", w.ant_name or "")
                if not m or w.ant_name in updated:
                    continue
                lane = int(m.group(1))
                assert w.wait_value == 16, (ins.name, w.ant_name, w.wait_value)
                assert lane in lane_sem, (ins.name, w.ant_name, lane_sem)
                u = lane_sem[lane]
                w.id = u.id
                w.ant_name = u.ant_name


def _get_nc():
    if "nc" not in _CACHED:
        _CACHED["nc"] = _build_nc()
    return _CACHED["nc"]


def _make_in_maps(x, W):
    x = np.ascontiguousarray(np.asarray(x, dtype=np.float32))
    W = np.ascontiguousarray(np.asarray(W, dtype=np.float32))
    xT = x.T.astype(np.float16)  # [D, B]
    in_maps = []
    for i in range(NCORES):
        Ws = W[i * CS:(i + 1) * CS].astype(np.float16)      # [CS, S, D]
        wT = Ws.transpose(2, 1, 0).reshape(D, R)            # [D, s*CS+c]
        xw = np.ascontiguousarray(np.concatenate([xT, wT], axis=1))
        in_maps.append({"xw": xw})
    return in_maps


def run(x, W, trace=False):
    nc = _get_nc()
    in_maps = _make_in_maps(x, W)
    res = run_bass_kernel_spmd(
        nc, in_maps, core_ids=list(range(NCORES)), trace=trace
    )
    shards = []
    for i in range(NCORES):
        o = np.asarray(res.results[i]["out"]).reshape(128, B)[0:CS, :]
        shards.append(o.astype(np.float32))
    out = np.concatenate([s.T for s in shards], axis=1)  # [B, C]
    return np.ascontiguousarray(out.astype(np.float32)), res


def kernel(x, W):
    out, _ = run(x, W, trace=False)
    return out
